# revision 21
# baseline (speedup 1.0000x reference)
"""Multi-head attention (QKV + softmax + out-proj + residual + LayerNorm)
as a Bass/Tile kernel for Trainium2, sharded over 8 NeuronCores.

Sharding: core c -> (batch b = c//2, query-half = c%2). Pure SPMD, no
collectives: each core computes K/V for its full batch sequence, Q only for
its 1024 query rows, all 16 heads, and its slice of both outputs.

Host-side prep (layout only): x is transposed + cast to bf16 once per batch
(xT, model-dim-major) so Q^T / K^T / V all come straight out of the
projection matmuls in the orientation the tensor engine needs; weights are
cast to bf16; the mask becomes an additive bias row (0 / -30000).
"""

import numpy as np
import ml_dtypes

BF16 = ml_dtypes.bfloat16

D_MODEL = 1024
N_HEADS = 16
B = 4
S = 2048
N_CORES = 8
MASK_NEG = -30000.0
EPS = 1e-5


def build_nc(S=2048, Q=1024, D=1024, H=16, AC=512):
    """Build the single-core Bass module. AC = attention psum chunk (<=1024)."""
    import os
    DBG_SKIP = set(os.environ.get("KDBG_SKIP", "").split(","))
    import concourse.bass as bass
    import concourse.mybir as mybir
    import concourse.tile as tile
    from concourse import bacc
    from concourse.masks import make_identity

    f32 = mybir.dt.float32
    bf16 = mybir.dt.bfloat16
    AL = mybir.AluOpType
    ACT = mybir.ActivationFunctionType

    P = 128
    dk = D // H
    assert dk == 64, "kernel assumes head dim 64"
    MT = D // P          # contraction tiles for projections
    DT = D // P          # d tiles
    KT = S // P          # key tiles
    QT = Q // P          # query tiles
    HP = H // 2          # head pairs
    CH = 512             # matmul free-dim chunk
    QCS = min(CH, Q)
    QC = Q // QCS
    KCS = min(CH, S)
    ECS = min(CH, D)
    EC = D // ECS
    AC = min(AC, S)
    AH = S // AC         # attention row chunk count (per q-tile)
    ACS = min(KCS, AC)   # score-matmul free chunk within an A-side psum
    BPAD = max(QCS, 512)  # pad scores^T psum so each head owns a full bank
    NSUB = max(D // 512, 1)  # bn_stats subgroups

    nc = bacc.Bacc("TRN2", target_bir_lowering=False, debug=False)

    xT_d = nc.dram_tensor("xT", [D, S], bf16, kind="ExternalInput")
    xTq_d = nc.dram_tensor("xTq", [D, Q], bf16, kind="ExternalInput")
    xres_d = nc.dram_tensor("xres", [Q, D], f32, kind="ExternalInput")
    wq_d = nc.dram_tensor("wq", [D, D], bf16, kind="ExternalInput")
    wk_d = nc.dram_tensor("wk", [D, D], bf16, kind="ExternalInput")
    wv_d = nc.dram_tensor("wv", [D, D], bf16, kind="ExternalInput")
    wo_d = nc.dram_tensor("wo", [D, D], bf16, kind="ExternalInput")
    bq_d = nc.dram_tensor("bq", [D], f32, kind="ExternalInput")
    bk_d = nc.dram_tensor("bk", [D], f32, kind="ExternalInput")
    bv_d = nc.dram_tensor("bv", [D], f32, kind="ExternalInput")
    mbf_d = nc.dram_tensor("maskb_f", [S], f32, kind="ExternalInput")
    m01_d = nc.dram_tensor("mask01_bf", [S], bf16, kind="ExternalInput")
    gamma_d = nc.dram_tensor("gamma", [D], f32, kind="ExternalInput")
    beta_d = nc.dram_tensor("beta", [D], f32, kind="ExternalInput")
    att_d = nc.dram_tensor("att", [H, Q, S], f32, kind="ExternalOutput")
    out_d = nc.dram_tensor("out", [Q, D], f32, kind="ExternalOutput")
    att = att_d.ap()
    out = out_d.ap()

    with tile.TileContext(nc) as tc:
        with tc.tile_pool(name="singles", bufs=1) as singles:
            # persistent SBUF state
            QT_s = singles.tile([P, DT, Q], bf16)     # Q^T (scaled, biased)
            KT_s = singles.tile([P, DT, S], bf16)     # K^T (biased)
            V_s = singles.tile([P, KT, D], bf16)      # V   (biased)
            ctxT_s = singles.tile([P, DT, Q], bf16)   # context^T (unnormalized)
            rowsums = singles.tile([P, H * QT], f32)
            recips = singles.tile([P, H * QT], f32)
            recips_bf = singles.tile([P, H * QT], bf16)
            mbp_s = singles.tile([P, KT], f32)        # mask bias, k on partitions
            m01_bc = singles.tile([P, S], bf16)       # 0/1 mask replicated
            ident_bf = singles.tile([P, P], bf16)
            zcol = singles.tile([1, P], bf16)
            zrow = singles.tile([1, CH], bf16)
            bq_s = singles.tile([P, MT], f32)
            bk_s = singles.tile([P, MT], f32)
            bv_bc = singles.tile([P, D], f32)
            gamma_bc = singles.tile([P, D], f32)
            beta_bc = singles.tile([P, D], f32)
            eps_s = singles.tile([P, 1], f32)

            nc.sync.dma_start(out=mbp_s, in_=mbf_d.ap().rearrange("(t p) -> p t", p=P))
            nc.sync.dma_start(out=m01_bc, in_=m01_d.ap()[None, :].to_broadcast((P, S)))
            nc.vector.memset(zcol, 0.0)
            nc.vector.memset(zrow, 0.0)
            make_identity(nc, ident_bf)
            nc.sync.dma_start(out=bq_s, in_=bq_d.ap().rearrange("(t p) -> p t", p=P))
            nc.sync.dma_start(out=bk_s, in_=bk_d.ap().rearrange("(t p) -> p t", p=P))
            nc.sync.dma_start(out=bv_bc, in_=bv_d.ap()[None, :].to_broadcast((P, D)))
            nc.sync.dma_start(
                out=gamma_bc, in_=gamma_d.ap()[None, :].to_broadcast((P, D))
            )
            nc.sync.dma_start(
                out=beta_bc, in_=beta_d.ap()[None, :].to_broadcast((P, D))
            )
            nc.vector.memset(eps_s, EPS)
            if DBG_SKIP & {"A", "PM"}:
                nc.vector.memset(rowsums, 1.0)
                nc.vector.memset(recips, 1.0)
            if "B" in DBG_SKIP:
                nc.vector.memset(ctxT_s, 0.0)

            # ---------------- Phase 1: projections ----------------
            with tc.tile_pool(name="ph1", bufs=1) as ph1, \
                 tc.tile_pool(name="wpool", bufs=2) as wpool, \
                 tc.tile_pool(name="ps1", bufs=4, space="PSUM") as ps1:
                xT_s = ph1.tile([P, MT, S], bf16, tag="xT")
                nc.sync.dma_start(
                    out=xT_s, in_=xT_d.ap().rearrange("(t p) s -> p t s", p=P)
                )
                xTq_s = ph1.tile([P, MT, Q], bf16, tag="xTq")
                nc.sync.dma_start(
                    out=xTq_s, in_=xTq_d.ap().rearrange("(t p) s -> p t s", p=P)
                )
                # weights cycle through a 2-slot pool, one section each
                wq_s = wpool.tile([P, MT, D], bf16, tag="w", name="wq_s")
                nc.sync.dma_start(
                    out=wq_s, in_=wq_d.ap().rearrange("(t p) d -> p t d", p=P)
                )
                for dt in range(DT):
                    # Q^T[d, q] (scaled by 1/sqrt(dk), + bq)
                    for qc in range(QC):
                        ps = ps1.tile([P, QCS], f32, tag="p1")
                        for mt in range(MT):
                            nc.tensor.matmul(
                                ps,
                                lhsT=wq_s[:, mt, dt * P:(dt + 1) * P],
                                rhs=xTq_s[:, mt, qc * QCS:(qc + 1) * QCS],
                                start=(mt == 0),
                                stop=(mt == MT - 1),
                            )
                        nc.vector.tensor_scalar(
                            out=QT_s[:, dt, qc * QCS:(qc + 1) * QCS],
                            in0=ps,
                            scalar1=bq_s[:, dt:dt + 1],
                            scalar2=1.0 / np.sqrt(dk),
                            op0=AL.add,
                            op1=AL.mult,
                        )
                wk_s = wpool.tile([P, MT, D], bf16, tag="w", name="wk_s")
                nc.sync.dma_start(
                    out=wk_s, in_=wk_d.ap().rearrange("(t p) d -> p t d", p=P)
                )
                for dt in range(DT):
                    # K^T[d, k] (+ bk)
                    for kc in range(S // KCS):
                        ps = ps1.tile([P, KCS], f32, tag="p1")
                        for mt in range(MT):
                            nc.tensor.matmul(
                                ps,
                                lhsT=wk_s[:, mt, dt * P:(dt + 1) * P],
                                rhs=xT_s[:, mt, kc * KCS:(kc + 1) * KCS],
                                start=(mt == 0),
                                stop=(mt == MT - 1),
                            )
                        nc.vector.tensor_scalar_add(
                            out=KT_s[:, dt, kc * KCS:(kc + 1) * KCS],
                            in0=ps,
                            scalar1=bk_s[:, dt:dt + 1],
                        )
                wv_s = wpool.tile([P, MT, D], bf16, tag="w", name="wv_s")
                nc.sync.dma_start(
                    out=wv_s, in_=wv_d.ap().rearrange("(t p) d -> p t d", p=P)
                )
                # V[k, d] (+ bv)
                for kt in range(KT):
                    for ec in range(EC):
                        ps = ps1.tile([P, ECS], f32, tag="p1")
                        for mt in range(MT):
                            nc.tensor.matmul(
                                ps,
                                lhsT=xT_s[:, mt, kt * P:(kt + 1) * P],
                                rhs=wv_s[:, mt, ec * ECS:(ec + 1) * ECS],
                                start=(mt == 0),
                                stop=(mt == MT - 1),
                            )
                        nc.vector.tensor_tensor(
                            V_s[:, kt, ec * ECS:(ec + 1) * ECS],
                            ps,
                            bv_bc[:, ec * ECS:(ec + 1) * ECS],
                            AL.add,
                        )

            # ---------------- Phase 2+3: attention ----------------
            with tc.tile_pool(name="psA", bufs=2, space="PSUM") as psA, \
                 tc.tile_pool(name="psB", bufs=1, space="PSUM") as psB, \
                 tc.tile_pool(name="psC", bufs=2, space="PSUM") as psC, \
                 tc.tile_pool(name="expq", bufs=2) as expq_pool, \
                 tc.tile_pool(name="attf", bufs=3) as attf_pool, \
                 tc.tile_pool(name="expt", bufs=3) as expt_pool, \
                 tc.tile_pool(name="sm", bufs=6) as small:
                for p in range(HP):
                    # ---- orientation A: scores[q, k] -> att output ----
                    # head pairs run concurrently in disjoint PE row groups
                    for qt in (range(0) if "A" in DBG_SKIP else range(QT)):
                        expq = [
                            expq_pool.tile([P, S], bf16, tag=f"expq{hh}",
                                           name=f"expq{hh}")
                            for hh in (0, 1)
                        ]
                        for half in range(AH):
                            psa = psA.tile([P, 2, AC], f32, tag="psa")
                            for kc in range(AC // ACS):
                                k0 = half * AC + kc * ACS
                                sl = slice(kc * ACS, (kc + 1) * ACS)
                                for hh in (0, 1):
                                    nc.tensor.matmul(
                                        psa[:, hh, sl],
                                        lhsT=QT_s[hh * 64:(hh + 1) * 64, p,
                                                  qt * P:(qt + 1) * P],
                                        rhs=KT_s[hh * 64:(hh + 1) * 64, p,
                                                 k0:k0 + ACS],
                                        start=True,
                                        stop=True,
                                    )
                            for hh in (0, 1):
                                nc.scalar.activation(
                                    out=expq[hh][:, half * AC:(half + 1) * AC],
                                    in_=psa[:, hh, :],
                                    func=ACT.Exp,
                                )
                        for hh in (0, 1):
                            h = 2 * p + hh
                            idx = h * QT + qt
                            nc.vector.scalar_tensor_tensor(
                                out=expq[hh],
                                in0=expq[hh],
                                scalar=1.0,
                                in1=m01_bc,
                                op0=AL.mult,
                                op1=AL.mult,
                                accum_out=rowsums[:, idx:idx + 1],
                            )
                            nc.vector.reciprocal(
                                out=recips[:, idx:idx + 1],
                                in_=rowsums[:, idx:idx + 1],
                            )
                            attf = attf_pool.tile([P, S], f32, tag="attf")
                            nc.vector.tensor_scalar_mul(
                                out=attf, in0=expq[hh],
                                scalar1=recips[:, idx:idx + 1],
                            )
                            nc.sync.dma_start(
                                out=att[h, qt * P:(qt + 1) * P, :], in_=attf
                            )

                    # ---- orientation B: scores^T[k, q] -> context^T ----
                    for qc in (range(0) if "B" in DBG_SKIP else range(QC)):
                        psc = psC.tile([P, QCS], f32, tag="psc")
                        # full-width zeroing matmul opens the accumulation
                        # region so the two 64-row col-group accumulators can
                        # share one psum bank
                        nc.tensor.matmul(
                            psc, lhsT=zcol[0:1, 0:P], rhs=zrow[0:1, 0:QCS],
                            start=True, stop=False,
                        )
                        for kt in range(KT):
                            psb = psB.tile([P, 2, BPAD], f32, tag="psb")
                            for hh in (0, 1):
                                nc.tensor.matmul(
                                    psb[:, hh, :QCS],
                                    lhsT=KT_s[hh * 64:(hh + 1) * 64, p,
                                              kt * P:(kt + 1) * P],
                                    rhs=QT_s[hh * 64:(hh + 1) * 64, p,
                                             qc * QCS:(qc + 1) * QCS],
                                    start=True,
                                    stop=True,
                                )
                            expt = expt_pool.tile([P, 2, QCS], bf16, tag="expt")
                            nc.scalar.activation(
                                out=expt,
                                in_=psb[:, :, :QCS],
                                func=ACT.Exp,
                                bias=mbp_s[:, kt:kt + 1],
                            )
                            for hh in (0, 1):
                                nc.tensor.matmul(
                                    psc[hh * 64:(hh + 1) * 64, :],
                                    lhsT=V_s[:, kt,
                                             (2 * p + hh) * 64:(2 * p + hh + 1) * 64],
                                    rhs=expt[:, hh, :],
                                    start=False,
                                    stop=False,
                                )
                        nc.tensor.matmul(
                            psc, lhsT=zcol[0:1, 0:P], rhs=zrow[0:1, 0:QCS],
                            start=False, stop=True,
                        )
                        nc.vector.tensor_copy(
                            out=ctxT_s[:, p, qc * QCS:(qc + 1) * QCS], in_=psc
                        )

                    # ---- normalize this pair's context by 1/rowsum ----
                    nc.vector.tensor_copy(
                        out=recips_bf[:, 2 * p * QT:(2 * p + 2) * QT],
                        in_=recips[:, 2 * p * QT:(2 * p + 2) * QT],
                    )
                    for qt in (range(0) if "PM" in DBG_SKIP else range(QT)):
                        pm = psC.tile([P, P], f32, tag="psc", name="pm")
                        nc.tensor.matmul(
                            pm, lhsT=zcol[0:1, 0:P], rhs=zrow[0:1, 0:P],
                            start=True, stop=False,
                        )
                        for hh in (0, 1):
                            j = (2 * p + hh) * QT + qt
                            nc.tensor.matmul(
                                pm[hh * 64:(hh + 1) * 64, :],
                                lhsT=recips_bf[:, j:j + 1].to_broadcast((P, 64)),
                                rhs=ident_bf,
                                start=False,
                                stop=False,
                            )
                        nc.tensor.matmul(
                            pm, lhsT=zcol[0:1, 0:P], rhs=zrow[0:1, 0:P],
                            start=False, stop=True,
                        )
                        nc.vector.tensor_tensor(
                            ctxT_s[:, p, qt * P:(qt + 1) * P],
                            ctxT_s[:, p, qt * P:(qt + 1) * P],
                            pm,
                            AL.mult,
                        )

            # ---------------- Phase 4: out-proj + residual + LN ----------------
            with tc.tile_pool(name="ph4", bufs=1) as ph4, \
                 tc.tile_pool(name="ps4", bufs=2, space="PSUM") as ps4, \
                 tc.tile_pool(name="ytile", bufs=3) as ypool, \
                 tc.tile_pool(name="xrt", bufs=3) as xrpool, \
                 tc.tile_pool(name="ln", bufs=4) as lnpool:
                wo_s = ph4.tile([P, MT, D], bf16, tag="wo")
                nc.sync.dma_start(
                    out=wo_s, in_=wo_d.ap().rearrange("(t p) d -> p t d", p=P)
                )
                for qt in range(QT):
                    y = ypool.tile([P, D], f32, tag="y")
                    for ec in range(EC):
                        ps = ps4.tile([P, ECS], f32, tag="p4")
                        for dt in range(DT):
                            nc.tensor.matmul(
                                ps,
                                lhsT=ctxT_s[:, dt, qt * P:(qt + 1) * P],
                                rhs=wo_s[:, dt, ec * ECS:(ec + 1) * ECS],
                                start=(dt == 0),
                                stop=(dt == DT - 1),
                            )
                        xr = xrpool.tile([P, ECS], f32, tag="xr")
                        nc.sync.dma_start(
                            out=xr,
                            in_=xres_d.ap()[qt * P:(qt + 1) * P,
                                            ec * ECS:(ec + 1) * ECS],
                        )
                        nc.vector.tensor_tensor(
                            y[:, ec * ECS:(ec + 1) * ECS], ps, xr, AL.add
                        )
                    # LayerNorm over D
                    stats = lnpool.tile([P, NSUB, 6], f32, tag="stats")
                    for sg in range(NSUB):
                        w = D // NSUB
                        nc.vector.bn_stats(
                            out=stats[:, sg, :], in_=y[:, sg * w:(sg + 1) * w]
                        )
                    mv = lnpool.tile([P, 2], f32, tag="mv")
                    nc.vector.bn_aggr(out=mv, in_=stats)
                    nc.scalar.activation(
                        out=mv[:, 1:2],
                        in_=mv[:, 1:2],
                        func=ACT.Sqrt,
                        bias=eps_s,
                    )
                    nc.vector.reciprocal(out=mv[:, 1:2], in_=mv[:, 1:2])
                    nc.vector.tensor_scalar(
                        out=y,
                        in0=y,
                        scalar1=mv[:, 0:1],
                        scalar2=mv[:, 1:2],
                        op0=AL.subtract,
                        op1=AL.mult,
                    )
                    nc.vector.tensor_tensor(y, y, gamma_bc, AL.mult)
                    nc.vector.tensor_tensor(y, y, beta_bc, AL.add)
                    nc.sync.dma_start(out=out[qt * P:(qt + 1) * P, :], in_=y)

    nc.compile()
    return nc


_NC_CACHE = {}


def _get_nc(key=(S, S // 2, D_MODEL, N_HEADS)):
    if key not in _NC_CACHE:
        _NC_CACHE[key] = build_nc(*key)
    return _NC_CACHE[key]


def make_core_inputs(x, mask, Wq, bq, Wk, bk, Wv, bv, Wo, bo, gamma, beta):
    """Host-side shard/layout prep. Returns list of per-core input dicts."""
    x = np.asarray(x, np.float32)
    mask = np.asarray(mask)
    f32 = lambda a: np.ascontiguousarray(np.asarray(a, np.float32))
    bfc = lambda a: np.ascontiguousarray(np.asarray(a, np.float32).astype(BF16))
    Q = x.shape[1] // 2
    wq_b, wk_b, wv_b, wo_b = bfc(Wq), bfc(Wk), bfc(Wv), bfc(Wo)
    bq_f, bk_f, bv_f = f32(bq), f32(bk), f32(bv)
    gamma_f, beta_f = f32(gamma), f32(beta)
    per_batch = []
    for b in range(x.shape[0]):
        xT = np.ascontiguousarray(x[b].astype(BF16).T)
        mb = np.where(mask[b, 0] == 0, np.float32(MASK_NEG), np.float32(0.0))
        m01 = (mask[b, 0] != 0).astype(np.float32)
        per_batch.append((xT, f32(mb), np.ascontiguousarray(m01.astype(BF16))))
    in_maps = []
    for c in range(N_CORES):
        b, half = c // 2, c % 2
        xT, mb_f, mb_b = per_batch[b]
        in_maps.append({
            "xT": xT,
            "xTq": np.ascontiguousarray(xT[:, half * Q:(half + 1) * Q]),
            "xres": f32(x[b, half * Q:(half + 1) * Q] + np.asarray(bo, np.float32)),
            "wq": wq_b, "wk": wk_b, "wv": wv_b, "wo": wo_b,
            "bq": bq_f, "bk": bk_f, "bv": bv_f,
            "maskb_f": mb_f, "mask01_bf": mb_b,
            "gamma": gamma_f, "beta": beta_f,
        })
    return in_maps


def run_on_cores(in_maps, trace=False, **kw):
    from concourse.bass_utils import run_bass_kernel_spmd

    nc = _get_nc()
    return run_bass_kernel_spmd(
        nc, in_maps, core_ids=list(range(N_CORES)), trace=trace, **kw
    )


def kernel(x, mask, Wq, bq, Wk, bk, Wv, bv, Wo, bo, gamma, beta):
    x = np.asarray(x, np.float32)
    Bn, Sn, Dn = x.shape
    Q = Sn // 2
    in_maps = make_core_inputs(x, mask, Wq, bq, Wk, bk, Wv, bv, Wo, bo, gamma, beta)
    res = run_on_cores(in_maps)
    out = np.empty((Bn, Sn, Dn), np.float32)
    att = np.empty((Bn, N_HEADS, Sn, Sn), np.float32)
    for c in range(N_CORES):
        b, half = c // 2, c % 2
        out[b, half * Q:(half + 1) * Q] = res.results[c]["out"]
        att[b, :, half * Q:(half + 1) * Q, :] = res.results[c]["att"]
    return out, att


# revision 28
# speedup vs baseline: 1.1094x; 1.1094x over previous
"""Multi-head attention (QKV + softmax + out-proj + residual + LayerNorm)
as a Bass/Tile kernel for Trainium2, sharded over 8 NeuronCores.

Sharding: core c -> (batch b = c//2, query-half = c%2). Pure SPMD, no
collectives: each core computes K/V for its full batch sequence, Q only for
its 1024 query rows, all 16 heads, and its slice of both outputs.

Host-side prep (layout only): x is transposed + cast to bf16 once per batch
(xT, model-dim-major) so Q^T / K^T / V all come straight out of the
projection matmuls in the orientation the tensor engine needs; weights are
cast to bf16; the mask becomes an additive bias row (0 / -30000).
"""

import numpy as np
import ml_dtypes

BF16 = ml_dtypes.bfloat16

D_MODEL = 1024
N_HEADS = 16
B = 4
S = 2048
N_CORES = 8
MASK_NEG = -30000.0
EPS = 1e-5


def build_nc(S=2048, Q=1024, D=1024, H=16, AC=512, SM=None):
    """Build the single-core Bass module. AC = attention psum chunk (<=1024)."""
    import os
    DBG_SKIP = set(os.environ.get("KDBG_SKIP", "").split(","))
    import concourse.bass as bass
    import concourse.mybir as mybir
    import concourse.tile as tile
    from concourse import bacc
    from concourse.masks import make_identity

    f32 = mybir.dt.float32
    bf16 = mybir.dt.bfloat16
    AL = mybir.AluOpType
    ACT = mybir.ActivationFunctionType

    P = 128
    dk = D // H
    assert dk == 64, "kernel assumes head dim 64"
    MT = D // P          # contraction tiles for projections
    DT = D // P          # d tiles
    KT = S // P          # key tiles
    QT = Q // P          # query tiles
    HP = H // 2          # head pairs
    CH = 512             # matmul free-dim chunk
    QCS = min(CH, Q)
    QC = Q // QCS
    KCS = min(CH, S)
    ECS = min(CH, D)
    EC = D // ECS
    AC = min(AC, S)
    AH = S // AC         # attention row chunk count (per q-tile)
    ACS = min(KCS, AC)   # score-matmul free chunk within an A-side psum
    BPAD = max(QCS, 512)  # pad scores^T psum so each head owns a full bank
    NSUB = max(D // 512, 1)  # bn_stats subgroups
    if SM is None:
        SM = S
    KTM = SM // P        # compacted (unmasked-only) key tiles for B side
    def _chunks(total, step=CH):
        return [(o, min(step, total - o)) for o in range(0, total, step)]

    nc = bacc.Bacc("TRN2", target_bir_lowering=False, debug=False)

    xT_d = nc.dram_tensor("xT", [D, S], bf16, kind="ExternalInput")
    xTq_d = nc.dram_tensor("xTq", [D, Q], bf16, kind="ExternalInput")
    xres_d = nc.dram_tensor("xres", [Q, D], f32, kind="ExternalInput")
    wq_d = nc.dram_tensor("wq", [D, D], bf16, kind="ExternalInput")
    wk_d = nc.dram_tensor("wk", [D, D], bf16, kind="ExternalInput")
    wv_d = nc.dram_tensor("wv", [D, D], bf16, kind="ExternalInput")
    wo_d = nc.dram_tensor("wo", [D, D], bf16, kind="ExternalInput")
    bq_d = nc.dram_tensor("bq", [D], f32, kind="ExternalInput")
    bk_d = nc.dram_tensor("bk", [D], f32, kind="ExternalInput")
    bv_d = nc.dram_tensor("bv", [D], f32, kind="ExternalInput")
    xTm_d = nc.dram_tensor("xTm", [D, SM], bf16, kind="ExternalInput")
    mbm_d = nc.dram_tensor("maskb_m", [SM], f32, kind="ExternalInput")
    m01_d = nc.dram_tensor("mask01_bf", [S], bf16, kind="ExternalInput")
    gamma_d = nc.dram_tensor("gamma", [D], f32, kind="ExternalInput")
    beta_d = nc.dram_tensor("beta", [D], f32, kind="ExternalInput")
    att_d = nc.dram_tensor("att", [H, Q, S], f32, kind="ExternalOutput")
    out_d = nc.dram_tensor("out", [Q, D], f32, kind="ExternalOutput")
    att = att_d.ap()
    out = out_d.ap()

    with tile.TileContext(nc) as tc:
        with tc.tile_pool(name="singles", bufs=1) as singles:
            # persistent SBUF state
            QT_s = singles.tile([P, DT, Q], bf16)     # Q^T (scaled, biased)
            KT_s = singles.tile([P, DT, S], bf16)     # K^T (biased, full)
            KTm_s = singles.tile([P, DT, SM], bf16)   # K^T (unmasked-compacted)
            V_s = singles.tile([P, KTM, D], bf16)     # V (unmasked-compacted)
            ctxT_s = singles.tile([P, DT, Q], bf16)   # context^T (unnormalized)
            rowsums = singles.tile([P, H * QT], f32)
            recips = singles.tile([P, H * QT], f32)
            recips_bf = singles.tile([P, H * QT], bf16)
            mbp_s = singles.tile([P, KTM], f32)       # pad bias, compacted k
            m01_bc = singles.tile([P, S], bf16)       # 0/1 mask replicated
            ident_bf = singles.tile([P, P], bf16)
            zcol = singles.tile([1, P], bf16)
            zrow = singles.tile([1, CH], bf16)
            bq_s = singles.tile([P, MT], f32)
            bk_s = singles.tile([P, MT], f32)
            bv_bc = singles.tile([P, D], f32)
            gamma_bc = singles.tile([P, D], f32)
            beta_bc = singles.tile([P, D], f32)
            eps_s = singles.tile([P, 1], f32)

            nc.sync.dma_start(out=mbp_s, in_=mbm_d.ap().rearrange("(t p) -> p t", p=P))
            nc.sync.dma_start(out=m01_bc, in_=m01_d.ap()[None, :].to_broadcast((P, S)))
            nc.vector.memset(zcol, 0.0)
            nc.vector.memset(zrow, 0.0)
            make_identity(nc, ident_bf)
            nc.sync.dma_start(out=bq_s, in_=bq_d.ap().rearrange("(t p) -> p t", p=P))
            nc.sync.dma_start(out=bk_s, in_=bk_d.ap().rearrange("(t p) -> p t", p=P))
            nc.sync.dma_start(out=bv_bc, in_=bv_d.ap()[None, :].to_broadcast((P, D)))
            nc.sync.dma_start(
                out=gamma_bc, in_=gamma_d.ap()[None, :].to_broadcast((P, D))
            )
            nc.sync.dma_start(
                out=beta_bc, in_=beta_d.ap()[None, :].to_broadcast((P, D))
            )
            nc.vector.memset(eps_s, EPS)
            if DBG_SKIP & {"A", "PM"}:
                nc.vector.memset(rowsums, 1.0)
                nc.vector.memset(recips, 1.0)
            if "B" in DBG_SKIP:
                nc.vector.memset(ctxT_s, 0.0)

            # ---------------- Phase 1: projections ----------------
            with tc.tile_pool(name="ph1", bufs=1) as ph1, \
                 tc.tile_pool(name="wpool", bufs=1) as wpool, \
                 tc.tile_pool(name="ps1", bufs=4, space="PSUM") as ps1:
                xT_s = ph1.tile([P, MT, S], bf16, tag="xT")
                nc.sync.dma_start(
                    out=xT_s, in_=xT_d.ap().rearrange("(t p) s -> p t s", p=P)
                )
                xTq_s = ph1.tile([P, MT, Q], bf16, tag="xTq")
                nc.sync.dma_start(
                    out=xTq_s, in_=xTq_d.ap().rearrange("(t p) s -> p t s", p=P)
                )
                xTm_s = ph1.tile([P, MT, SM], bf16, tag="xTm")
                nc.sync.dma_start(
                    out=xTm_s, in_=xTm_d.ap().rearrange("(t p) s -> p t s", p=P)
                )
                # weights cycle through a 2-slot pool, one section each
                wq_s = wpool.tile([P, MT, D], bf16, tag="w", name="wq_s")
                nc.sync.dma_start(
                    out=wq_s, in_=wq_d.ap().rearrange("(t p) d -> p t d", p=P)
                )
                for dt in range(DT):
                    # Q^T[d, q] (scaled by 1/sqrt(dk), + bq)
                    for qc in range(QC):
                        ps = ps1.tile([P, QCS], f32, tag="p1")
                        for mt in range(MT):
                            nc.tensor.matmul(
                                ps,
                                lhsT=wq_s[:, mt, dt * P:(dt + 1) * P],
                                rhs=xTq_s[:, mt, qc * QCS:(qc + 1) * QCS],
                                start=(mt == 0),
                                stop=(mt == MT - 1),
                            )
                        nc.vector.tensor_scalar(
                            out=QT_s[:, dt, qc * QCS:(qc + 1) * QCS],
                            in0=ps,
                            scalar1=bq_s[:, dt:dt + 1],
                            scalar2=1.0 / np.sqrt(dk),
                            op0=AL.add,
                            op1=AL.mult,
                        )
                wk_s = wpool.tile([P, MT, D], bf16, tag="w", name="wk_s")
                nc.sync.dma_start(
                    out=wk_s, in_=wk_d.ap().rearrange("(t p) d -> p t d", p=P)
                )
                for dt in range(DT):
                    # K^T[d, k] (+ bk) — full, for the att orientation
                    for kc in range(S // KCS):
                        ps = ps1.tile([P, KCS], f32, tag="p1")
                        for mt in range(MT):
                            nc.tensor.matmul(
                                ps,
                                lhsT=wk_s[:, mt, dt * P:(dt + 1) * P],
                                rhs=xT_s[:, mt, kc * KCS:(kc + 1) * KCS],
                                start=(mt == 0),
                                stop=(mt == MT - 1),
                            )
                        nc.vector.tensor_scalar_add(
                            out=KT_s[:, dt, kc * KCS:(kc + 1) * KCS],
                            in0=ps,
                            scalar1=bk_s[:, dt:dt + 1],
                        )
                    # K^T compacted to unmasked keys, for the context side
                    for o, w in _chunks(SM):
                        ps = ps1.tile([P, KCS], f32, tag="p1")
                        for mt in range(MT):
                            nc.tensor.matmul(
                                ps[:, :w],
                                lhsT=wk_s[:, mt, dt * P:(dt + 1) * P],
                                rhs=xTm_s[:, mt, o:o + w],
                                start=(mt == 0),
                                stop=(mt == MT - 1),
                            )
                        nc.vector.tensor_scalar_add(
                            out=KTm_s[:, dt, o:o + w],
                            in0=ps[:, :w],
                            scalar1=bk_s[:, dt:dt + 1],
                        )
                wv_s = wpool.tile([P, MT, D], bf16, tag="w", name="wv_s")
                nc.sync.dma_start(
                    out=wv_s, in_=wv_d.ap().rearrange("(t p) d -> p t d", p=P)
                )
                # V[k, d] (+ bv) — unmasked-compacted
                for kt in range(KTM):
                    for ec in range(EC):
                        ps = ps1.tile([P, ECS], f32, tag="p1")
                        for mt in range(MT):
                            nc.tensor.matmul(
                                ps,
                                lhsT=xTm_s[:, mt, kt * P:(kt + 1) * P],
                                rhs=wv_s[:, mt, ec * ECS:(ec + 1) * ECS],
                                start=(mt == 0),
                                stop=(mt == MT - 1),
                            )
                        nc.vector.tensor_tensor(
                            V_s[:, kt, ec * ECS:(ec + 1) * ECS],
                            ps,
                            bv_bc[:, ec * ECS:(ec + 1) * ECS],
                            AL.add,
                        )

            # ---------------- Phase 2+3: attention ----------------
            with tc.tile_pool(name="psA", bufs=2, space="PSUM") as psA, \
                 tc.tile_pool(name="psB", bufs=1, space="PSUM") as psB, \
                 tc.tile_pool(name="psC", bufs=2, space="PSUM") as psC, \
                 tc.tile_pool(name="expq", bufs=2) as expq_pool, \
                 tc.tile_pool(name="attf", bufs=3) as attf_pool, \
                 tc.tile_pool(name="expt", bufs=3) as expt_pool, \
                 tc.tile_pool(name="sm", bufs=6) as small:
                for p in range(HP):
                    # ---- orientation A: scores[q, k] -> att output ----
                    # head pairs run concurrently in disjoint PE row groups
                    for qt in (range(0) if "A" in DBG_SKIP else range(QT)):
                        expq = expq_pool.tile([P, 2, S], bf16, tag="expq")
                        for half in range(AH):
                            psa = psA.tile([P, 2, AC], f32, tag="psa")
                            for kc in range(AC // ACS):
                                k0 = half * AC + kc * ACS
                                sl = slice(kc * ACS, (kc + 1) * ACS)
                                for hh in (0, 1):
                                    nc.tensor.matmul(
                                        psa[:, hh, sl],
                                        lhsT=QT_s[hh * 64:(hh + 1) * 64, p,
                                                  qt * P:(qt + 1) * P],
                                        rhs=KT_s[hh * 64:(hh + 1) * 64, p,
                                                 k0:k0 + ACS],
                                        start=True,
                                        stop=True,
                                    )
                            nc.scalar.activation(
                                out=expq[:, :, half * AC:(half + 1) * AC],
                                in_=psa,
                                func=ACT.Exp,
                            )
                        for hh in (0, 1):
                            h = 2 * p + hh
                            idx = h * QT + qt
                            nc.vector.scalar_tensor_tensor(
                                out=expq[:, hh, :],
                                in0=expq[:, hh, :],
                                scalar=1.0,
                                in1=m01_bc,
                                op0=AL.mult,
                                op1=AL.mult,
                                accum_out=rowsums[:, idx:idx + 1],
                            )
                            nc.vector.reciprocal(
                                out=recips[:, idx:idx + 1],
                                in_=rowsums[:, idx:idx + 1],
                            )
                            attf = attf_pool.tile([P, S], f32, tag="attf")
                            nc.vector.tensor_scalar_mul(
                                out=attf, in0=expq[:, hh, :],
                                scalar1=recips[:, idx:idx + 1],
                            )
                            nc.sync.dma_start(
                                out=att[h, qt * P:(qt + 1) * P, :], in_=attf
                            )

                    # ---- orientation B: scores^T[k, q] -> context^T ----
                    for qc in (range(0) if "B" in DBG_SKIP else range(QC)):
                        psc = psC.tile([P, QCS], f32, tag="psc")
                        # full-width zeroing matmul opens the accumulation
                        # region so the two 64-row col-group accumulators can
                        # share one psum bank
                        nc.tensor.matmul(
                            psc, lhsT=zcol[0:1, 0:P], rhs=zrow[0:1, 0:QCS],
                            start=True, stop=False,
                        )
                        for kt in range(KTM):
                            psb = psB.tile([P, 2, BPAD], f32, tag="psb")
                            for hh in (0, 1):
                                nc.tensor.matmul(
                                    psb[:, hh, :QCS],
                                    lhsT=KTm_s[hh * 64:(hh + 1) * 64, p,
                                               kt * P:(kt + 1) * P],
                                    rhs=QT_s[hh * 64:(hh + 1) * 64, p,
                                             qc * QCS:(qc + 1) * QCS],
                                    start=True,
                                    stop=True,
                                )
                            expt = expt_pool.tile([P, 2, QCS], bf16, tag="expt")
                            nc.scalar.activation(
                                out=expt,
                                in_=psb[:, :, :QCS],
                                func=ACT.Exp,
                                bias=mbp_s[:, kt:kt + 1],
                            )
                            for hh in (0, 1):
                                nc.tensor.matmul(
                                    psc[hh * 64:(hh + 1) * 64, :],
                                    lhsT=V_s[:, kt,
                                             (2 * p + hh) * 64:(2 * p + hh + 1) * 64],
                                    rhs=expt[:, hh, :],
                                    start=False,
                                    stop=False,
                                )
                        nc.tensor.matmul(
                            psc, lhsT=zcol[0:1, 0:P], rhs=zrow[0:1, 0:QCS],
                            start=False, stop=True,
                        )
                        nc.vector.tensor_copy(
                            out=ctxT_s[:, p, qc * QCS:(qc + 1) * QCS], in_=psc
                        )

                    # ---- normalize this pair's context by 1/rowsum ----
                    nc.vector.tensor_copy(
                        out=recips_bf[:, 2 * p * QT:(2 * p + 2) * QT],
                        in_=recips[:, 2 * p * QT:(2 * p + 2) * QT],
                    )
                    for qt in (range(0) if "PM" in DBG_SKIP else range(QT)):
                        pm = psC.tile([P, P], f32, tag="psc", name="pm")
                        nc.tensor.matmul(
                            pm, lhsT=zcol[0:1, 0:P], rhs=zrow[0:1, 0:P],
                            start=True, stop=False,
                        )
                        for hh in (0, 1):
                            j = (2 * p + hh) * QT + qt
                            nc.tensor.matmul(
                                pm[hh * 64:(hh + 1) * 64, :],
                                lhsT=recips_bf[:, j:j + 1].to_broadcast((P, 64)),
                                rhs=ident_bf,
                                start=False,
                                stop=False,
                            )
                        nc.tensor.matmul(
                            pm, lhsT=zcol[0:1, 0:P], rhs=zrow[0:1, 0:P],
                            start=False, stop=True,
                        )
                        nc.vector.tensor_tensor(
                            ctxT_s[:, p, qt * P:(qt + 1) * P],
                            ctxT_s[:, p, qt * P:(qt + 1) * P],
                            pm,
                            AL.mult,
                        )

            # ---------------- Phase 4: out-proj + residual + LN ----------------
            with tc.tile_pool(name="ph4", bufs=1) as ph4, \
                 tc.tile_pool(name="ps4", bufs=2, space="PSUM") as ps4, \
                 tc.tile_pool(name="ytile", bufs=3) as ypool, \
                 tc.tile_pool(name="xrt", bufs=3) as xrpool, \
                 tc.tile_pool(name="ln", bufs=4) as lnpool:
                wo_s = ph4.tile([P, MT, D], bf16, tag="wo")
                nc.sync.dma_start(
                    out=wo_s, in_=wo_d.ap().rearrange("(t p) d -> p t d", p=P)
                )
                for qt in range(QT):
                    y = ypool.tile([P, D], f32, tag="y")
                    for ec in range(EC):
                        ps = ps4.tile([P, ECS], f32, tag="p4")
                        for dt in range(DT):
                            nc.tensor.matmul(
                                ps,
                                lhsT=ctxT_s[:, dt, qt * P:(qt + 1) * P],
                                rhs=wo_s[:, dt, ec * ECS:(ec + 1) * ECS],
                                start=(dt == 0),
                                stop=(dt == DT - 1),
                            )
                        xr = xrpool.tile([P, ECS], f32, tag="xr")
                        nc.sync.dma_start(
                            out=xr,
                            in_=xres_d.ap()[qt * P:(qt + 1) * P,
                                            ec * ECS:(ec + 1) * ECS],
                        )
                        nc.vector.tensor_tensor(
                            y[:, ec * ECS:(ec + 1) * ECS], ps, xr, AL.add
                        )
                    # LayerNorm over D
                    stats = lnpool.tile([P, NSUB, 6], f32, tag="stats")
                    for sg in range(NSUB):
                        w = D // NSUB
                        nc.vector.bn_stats(
                            out=stats[:, sg, :], in_=y[:, sg * w:(sg + 1) * w]
                        )
                    mv = lnpool.tile([P, 2], f32, tag="mv")
                    nc.vector.bn_aggr(out=mv, in_=stats)
                    nc.scalar.activation(
                        out=mv[:, 1:2],
                        in_=mv[:, 1:2],
                        func=ACT.Sqrt,
                        bias=eps_s,
                    )
                    nc.vector.reciprocal(out=mv[:, 1:2], in_=mv[:, 1:2])
                    nc.vector.tensor_scalar(
                        out=y,
                        in0=y,
                        scalar1=mv[:, 0:1],
                        scalar2=mv[:, 1:2],
                        op0=AL.subtract,
                        op1=AL.mult,
                    )
                    nc.vector.tensor_tensor(y, y, gamma_bc, AL.mult)
                    nc.vector.tensor_tensor(y, y, beta_bc, AL.add)
                    nc.sync.dma_start(out=out[qt * P:(qt + 1) * P, :], in_=y)

    nc.compile()
    return nc


_NC_CACHE = {}


def _get_nc(key):
    if key not in _NC_CACHE:
        _NC_CACHE[key] = build_nc(*key)
    return _NC_CACHE[key]


def make_core_inputs(x, mask, Wq, bq, Wk, bk, Wv, bv, Wo, bo, gamma, beta):
    """Host-side shard/layout prep. Returns list of per-core input dicts."""
    x = np.asarray(x, np.float32)
    mask = np.asarray(mask)
    f32 = lambda a: np.ascontiguousarray(np.asarray(a, np.float32))
    bfc = lambda a: np.ascontiguousarray(np.asarray(a, np.float32).astype(BF16))
    Q = x.shape[1] // 2
    wq_b, wk_b, wv_b, wo_b = bfc(Wq), bfc(Wk), bfc(Wv), bfc(Wo)
    bq_f, bk_f, bv_f = f32(bq), f32(bk), f32(bv)
    gamma_f, beta_f = f32(gamma), f32(beta)
    counts = [int((mask[b, 0] != 0).sum()) for b in range(x.shape[0])]
    SM = max(128, -(-max(counts) // 128) * 128)
    per_batch = []
    for b in range(x.shape[0]):
        xT = np.ascontiguousarray(x[b].astype(BF16).T)
        m01 = (mask[b, 0] != 0).astype(np.float32)
        idx = np.nonzero(mask[b, 0])[0]
        xTm = np.zeros((xT.shape[0], SM), dtype=BF16)
        xTm[:, :len(idx)] = xT[:, idx]
        mbm = np.full(SM, np.float32(MASK_NEG), np.float32)
        mbm[:len(idx)] = 0.0
        per_batch.append((xT, np.ascontiguousarray(xTm), f32(mbm),
                          np.ascontiguousarray(m01.astype(BF16))))
    in_maps = []
    for c in range(N_CORES):
        b, half = c // 2, c % 2
        xT, xTm, mbm, m01b = per_batch[b]
        in_maps.append({
            "xT": xT,
            "xTq": np.ascontiguousarray(xT[:, half * Q:(half + 1) * Q]),
            "xTm": xTm,
            "xres": f32(x[b, half * Q:(half + 1) * Q] + np.asarray(bo, np.float32)),
            "wq": wq_b, "wk": wk_b, "wv": wv_b, "wo": wo_b,
            "bq": bq_f, "bk": bk_f, "bv": bv_f,
            "maskb_m": mbm, "mask01_bf": m01b,
            "gamma": gamma_f, "beta": beta_f,
        })
    return in_maps, SM


def run_on_cores(in_maps, SM, trace=False, **kw):
    from concourse.bass_utils import run_bass_kernel_spmd

    nc = _get_nc((S, S // 2, D_MODEL, N_HEADS, 512, SM))
    return run_bass_kernel_spmd(
        nc, in_maps, core_ids=list(range(N_CORES)), trace=trace, **kw
    )


def kernel(x, mask, Wq, bq, Wk, bk, Wv, bv, Wo, bo, gamma, beta):
    x = np.asarray(x, np.float32)
    Bn, Sn, Dn = x.shape
    Q = Sn // 2
    in_maps, SM = make_core_inputs(x, mask, Wq, bq, Wk, bk, Wv, bv, Wo, bo, gamma, beta)
    res = run_on_cores(in_maps, SM)
    out = np.empty((Bn, Sn, Dn), np.float32)
    att = np.empty((Bn, N_HEADS, Sn, Sn), np.float32)
    for c in range(N_CORES):
        b, half = c // 2, c % 2
        out[b, half * Q:(half + 1) * Q] = res.results[c]["out"]
        att[b, :, half * Q:(half + 1) * Q, :] = res.results[c]["att"]
    return out, att
